# revision 2
# baseline (speedup 1.0000x reference)
"""Trainium2 Bass kernel for ExtendedGCN — v5 (indirect-DMA gathers).

Math (reference):
    agg(F) = D^-1/2 (A+I) D^-1/2 F,  deg = in-deg + 1
    Z1 = agg(x) @ W1 + b1 ; H1 = relu(Z1)
    Z2 = agg(H1) @ W2 + b2
    Z3 = agg(Z2) @ W3 + b3 ; out = softmax(Z3)

Identities (dinv = deg^-1/2; row scaling commutes with right matmul):
    T1 = (dinv*x) @ W1            [n,128] bf16, built locally on every core
                                  from replicated x^T (no AllGather)
    S1 = rowsum_{N(i)+i} T1 ;  X2 = relu(dinv^2*S1 + dinv*b1)
    T2 = X2 @ W2                  [n,64] bf16, AllGathered in 3 chunks
    S2 = rowsum T2             ;  X3 = dinv^2*S2 + dinv*b2
    T3 = X3 @ W3                  [n,16] bf16, AllGathered in 3 chunks
    S3 = rowsum T3             ;  Z3 = dinv*S3 + b3 ; out = softmax(Z3)

Nodes are partitioned across 8 cores (dst-owner), sorted by degree so the
128-node blocks have uniform slot counts (~2% padding). Pad nodes carry
dinv=0 so every pad table row is exactly zero; one pad row serves as the
gather target for padding slots. Gathers are per-(block, slot) indirect
DMAs ([128,1] row offsets). AllGathers are chunked (3 row-ranges, separate
DRAM tensors) so they overlap trailing block compute.
"""

import sys

sys.path.insert(0, "/opt/trn_rl_repo")

import numpy as np

N_CORES = 8
P = 128
NCHUNK = 1
ABLATE = ""   # "ag" skips collectives, "gather" skips gathers (timing only)


# --------------------------------------------------------------------------
# Host-side graph preprocessing
# --------------------------------------------------------------------------
def preprocess5(edge_index, n_nodes, n_cores=N_CORES):
    src = np.asarray(edge_index[0]).astype(np.int64)
    dst = np.asarray(edge_index[1]).astype(np.int64)
    n = n_nodes
    deg = np.bincount(dst, minlength=n).astype(np.int64) + 1

    chunk = P * n_cores
    n_pad = ((n + chunk - 1) // chunk) * chunk
    J = n_pad // chunk
    assert n_pad > n, "need at least one pad node for the zero row"

    order = np.argsort(-deg, kind="stable")
    rank = np.empty(n, np.int64)
    rank[order] = np.arange(n)

    deg_pad = np.zeros(n_pad, np.int64)
    deg_pad[rank] = deg
    K_u = [int(deg_pad[jj * chunk:(jj + 1) * chunk].max()) for jj in range(J)]
    S = int(np.sum(K_u))
    off = np.concatenate([[0], np.cumsum(K_u)[:-1]]).astype(np.int64)

    # AG-chunk cuts: contiguous jj ranges, slot-balanced
    target = S / NCHUNK
    cuts = [0]
    acc = 0
    for jj in range(J):
        acc += K_u[jj]
        if acc >= target * len(cuts) and len(cuts) < NCHUNK:
            cuts.append(jj + 1)
    while len(cuts) < NCHUNK:
        cuts.append(J)
    cuts.append(J)
    jj_cut = np.asarray(cuts, np.int64)
    nblkc = np.diff(jj_cut)
    chunk_row_base = np.concatenate([[0], np.cumsum(nblkc * n_cores * P)])
    seg_of_jj = np.zeros(J, np.int64)
    for c in range(NCHUNK):
        seg_of_jj[jj_cut[c]:jj_cut[c + 1]] = c

    def row_of_rank(r):
        r = np.asarray(r)
        g = r // P
        p = r % P
        core = g % n_cores
        jj = g // n_cores
        c = seg_of_jj[jj]
        return (chunk_row_base[c] + core * (nblkc[c] * P)
                + (jj - jj_cut[c]) * P + p)

    row_of_node = row_of_rank(rank)
    ZROW = int(row_of_rank(n_pad - 1))  # last pad's row: always zero

    # (core, jj, p) decode
    def decode_row(rr):
        rr = np.asarray(rr)
        c = np.searchsorted(chunk_row_base, rr, side="right") - 1
        offr = rr - chunk_row_base[c]
        span = nblkc[c] * P
        core = offr // span
        jj = jj_cut[c] + (offr % span) // P
        p = rr % P
        return core, jj, p

    core_n, jj_n, p_n = decode_row(row_of_node)

    # slot lists: gidx[core, p, off[jj]+s] = table row of s-th contributor
    gidx = np.full((n_cores, P, S), ZROW, dtype=np.int32)
    # self-loops at slot 0
    gidx[core_n, p_n, off[jj_n]] = row_of_node.astype(np.int32)
    # edges at slots 1..deg-1, grouped by dst
    er = rank[dst]
    eorder = np.argsort(er, kind="stable")
    er_s = er[eorder]
    src_rows = row_of_node[src[eorder]].astype(np.int32)
    cnt = np.bincount(er_s, minlength=n_pad)
    start = np.concatenate([[0], np.cumsum(cnt)[:-1]])
    slot = np.arange(len(er_s)) - start[er_s] + 1
    dcore = core_n[rank == er_s] if False else None
    dr = row_of_node[dst[eorder]]
    dc, dj, dp = decode_row(dr)
    gidx[dc, dp, off[dj] + slot] = src_rows

    # dinv arrays (pads = 0)
    node_of_row = np.full(n_pad, -1, np.int64)
    node_of_row[row_of_node] = np.arange(n)
    dinv_row = np.zeros(n_pad, np.float64)
    m = node_of_row >= 0
    dinv_row[m] = deg[node_of_row[m]] ** -0.5
    dinv2_row = dinv_row ** 2

    NBLK = n_pad // P
    dinv1_all = dinv_row.reshape(NBLK, P).T.astype(np.float32).copy()

    core_r, jj_r, p_r = decode_row(np.arange(n_pad))
    tbl_row_of = np.zeros((n_cores, J * P), np.int64)
    tbl_row_of[core_r, jj_r * P + p_r] = np.arange(n_pad)
    dinv1_own = np.zeros((n_cores, P, J), np.float32)
    dinv2_own = np.zeros((n_cores, P, J), np.float32)
    dinvr_own = np.zeros((n_cores, 1, J * P), np.float32)
    for c in range(n_cores):
        dv = dinv_row[tbl_row_of[c]]
        dinv1_own[c] = dv.reshape(J, P).T
        dinv2_own[c] = dinv2_row[tbl_row_of[c]].reshape(J, P).T
        dinvr_own[c, 0] = dv

    return dict(
        n_pad=n_pad, J=J, S=S, K_u=K_u, off=off, jj_cut=jj_cut,
        nblkc=nblkc, chunk_row_base=chunk_row_base,
        gidx=gidx, dinv1_all=dinv1_all,
        dinv1_own=dinv1_own, dinv2_own=dinv2_own, dinvr_own=dinvr_own,
        row_of_node=row_of_node, tbl_row_of=tbl_row_of, ZROW=ZROW,
    )


# --------------------------------------------------------------------------
# Bass program
# --------------------------------------------------------------------------
def build_bass5(pre, D0, D1, D2, D3, n_cores=N_CORES):
    import concourse.bass as bass
    import concourse.bacc as bacc
    import concourse.mybir as mybir
    import concourse.tile as tile
    from concourse.masks import make_identity

    f32 = mybir.dt.float32
    bf16 = mybir.dt.bfloat16
    i32 = mybir.dt.int32

    J = pre["J"]
    n_pad = pre["n_pad"]
    S = pre["S"]
    K_u = pre["K_u"]
    off = pre["off"]
    jj_cut = pre["jj_cut"]
    nblkc = pre["nblkc"]
    chunk_row_base = pre["chunk_row_base"]
    NBLK = n_pad // P
    Kmax = int(max(K_u))
    rg = [list(range(n_cores))]
    add = mybir.AluOpType.add

    nc = bacc.Bacc("TRN2", target_bir_lowering=False, num_devices=n_cores)

    x_T = nc.dram_tensor("x_T", [P, n_pad], bf16, kind="ExternalInput")
    gidx = nc.dram_tensor("gidx", [P, S], i32, kind="ExternalInput")
    d1all = nc.dram_tensor("d1all", [P, NBLK], f32, kind="ExternalInput")
    d1own = nc.dram_tensor("d1own", [P, J], f32, kind="ExternalInput")
    d2own = nc.dram_tensor("d2own", [P, J], f32, kind="ExternalInput")
    drown = nc.dram_tensor("drown", [1, J * P], bf16, kind="ExternalInput")
    W1 = nc.dram_tensor("W1", [D0, D1], f32, kind="ExternalInput")
    W2 = nc.dram_tensor("W2", [D1, D2], f32, kind="ExternalInput")
    W3 = nc.dram_tensor("W3", [D2, D3], f32, kind="ExternalInput")
    b1 = nc.dram_tensor("b1", [1, D1], f32, kind="ExternalInput")
    b2 = nc.dram_tensor("b2", [1, D2], f32, kind="ExternalInput")
    b3 = nc.dram_tensor("b3", [1, D3], f32, kind="ExternalInput")
    out = nc.dram_tensor("out", [J * P, D3], f32, kind="ExternalOutput")

    t1 = nc.dram_tensor("t1", [n_pad, D1], bf16)
    sl2 = [nc.dram_tensor(f"sl2_{c}", [int(nblkc[c]) * P, D2], bf16)
           for c in range(NCHUNK)]
    sl3 = [nc.dram_tensor(f"sl3_{c}", [int(nblkc[c]) * P, D3], bf16)
           for c in range(NCHUNK)]
    t2 = nc.dram_tensor("t2", [n_pad, D2], bf16, addr_space="Shared")
    t3 = nc.dram_tensor("t3", [n_pad, D3], bf16, addr_space="Shared")

    chunk_of_jj = np.zeros(J, np.int64)
    for c in range(NCHUNK):
        chunk_of_jj[jj_cut[c]:jj_cut[c + 1]] = c

    with tile.TileContext(nc) as tc:
        with (
            tc.tile_pool(name="const", bufs=1) as cpool,
            tc.tile_pool(name="gpool", bufs=3) as gpool,
            tc.tile_pool(name="xph", bufs=2) as xpool,
            tc.tile_pool(name="work", bufs=2) as wpool,
            tc.tile_pool(name="batch", bufs=2) as bpool,
            tc.tile_pool(name="small", bufs=4) as mpool,
            tc.tile_pool(name="psA", bufs=4, space="PSUM") as ppa,
            tc.tile_pool(name="psB", bufs=4, space="PSUM") as ppb,
        ):
            identf = cpool.tile([P, P], f32)
            make_identity(nc, identf[:, :])
            gidx_sb = cpool.tile([P, S], i32)
            nc.sync.dma_start(out=gidx_sb[:, :], in_=gidx[:, :])

            def load_cast(dram, rows, cols, dt_out, nm):
                tmp = wpool.tile([rows, cols], f32, tag="ldcast", name=f"ld_{nm}")
                nc.sync.dma_start(out=tmp[:, :], in_=dram[:, :])
                dst = cpool.tile([rows, cols], dt_out, name=nm)
                nc.vector.tensor_copy(out=dst[:, :], in_=tmp[:, :])
                return dst

            W1b = load_cast(W1, D0, D1, bf16, "W1b")
            W2b = load_cast(W2, D1, D2, bf16, "W2b")
            W3b = load_cast(W3, D2, D3, bf16, "W3b")
            b1b = load_cast(b1, 1, D1, bf16, "b1b")
            b2b = load_cast(b2, 1, D2, bf16, "b2b")
            b3b = load_cast(b3, 1, D3, bf16, "b3b")
            drownb = cpool.tile([1, J * P], bf16)
            nc.sync.dma_start(out=drownb[:, :], in_=drown[:, :])
            onesb = cpool.tile([1, P], bf16)
            nc.gpsimd.memset(onesb[:, :], 1.0)

            d1all_sb = cpool.tile([P, NBLK], f32)
            nc.sync.dma_start(out=d1all_sb[:, :], in_=d1all[:, :])
            d1own_sb = cpool.tile([P, J], f32)
            nc.sync.dma_start(out=d1own_sb[:, :], in_=d1own[:, :])
            d2own_sb = cpool.tile([P, J], f32)
            nc.sync.dma_start(out=d2own_sb[:, :], in_=d2own[:, :])

            # ---- phase 0: T1 = (dinv * x) @ W1 for ALL rows, locally ----
            XB = 8
            for it in range(NBLK // XB):
                xt = xpool.tile([P, XB * P], bf16, tag="xt")
                nc.sync.dma_start(
                    out=xt[:, :], in_=x_T[:, it * XB * P:(it + 1) * XB * P]
                )
                y1 = xpool.tile([P, XB, D1], bf16, tag="y1")
                for b in range(XB):
                    ps = ppa.tile([P, P], f32, tag="pa")
                    nc.tensor.matmul(
                        out=ps[:, :], lhsT=xt[:, b * P:(b + 1) * P],
                        rhs=W1b[:, :], start=True, stop=True,
                    )
                    g = it * XB + b
                    nc.vector.tensor_scalar_mul(
                        out=y1[:, b, :], in0=ps[:, :],
                        scalar1=d1all_sb[:, g:g + 1],
                    )
                nc.sync.dma_start(
                    out=t1[it * XB * P:(it + 1) * XB * P, :].rearrange(
                        "(b p) c -> p b c", p=P
                    ),
                    in_=y1[:, :, :],
                )

            def tree_add(G, K, Din):
                """Sum G[:, 0:K, :Din] -> returns AP [P, Din] (f32)."""
                Hxr = wpool.tile([P, Kmax // 2 + 1, 128], f32, tag="hx")
                Hx = Hxr[:, :, :Din]
                m = K // 2
                if m > 0:
                    nc.vector.tensor_tensor(
                        out=Hx[:, :m, :], in0=G[:, :m, :Din],
                        in1=G[:, m:2 * m, :Din], op=add,
                    )
                if K % 2:
                    nc.vector.tensor_copy(
                        out=Hx[:, m:m + 1, :], in_=G[:, K - 1:K, :Din],
                    )
                k = m + (K % 2)
                while k > 1:
                    m = k // 2
                    nc.vector.tensor_tensor(
                        out=Hx[:, :m, :], in0=Hx[:, :m, :],
                        in1=Hx[:, k - m:k, :], op=add,
                    )
                    k -= m
                return Hx[:, 0, :]

            def layer(table, Din, kind):
                """kind 1: relu+W2 -> sl2 ; kind 2: affine+W3 -> sl3 ;
                kind 3: dinv*S+b3, softmax -> out."""
                nb = 8 if kind == 3 else 4
                Dout = {1: D2, 2: D3, 3: D3}[kind]
                ybatch = None
                y0 = ycnt = 0

                def flush(yb, jj0, cnt):
                    if cnt == 0:
                        return
                    if kind == 3:
                        dst = out[jj0 * P:(jj0 + cnt) * P, :]
                    else:
                        sl = sl2 if kind == 1 else sl3
                        c = int(chunk_of_jj[jj0])
                        base = jj0 - int(jj_cut[c])
                        dst = sl[c][base * P:(base + cnt) * P, :]
                    nc.sync.dma_start(
                        out=dst.rearrange("(b p) c -> p b c", p=P),
                        in_=yb[:, :cnt, :],
                    )

                for jj in range(J):
                    K = int(K_u[jj])
                    o = int(off[jj])
                    G = gpool.tile([P, Kmax, 128], bf16, tag="g")
                    if ABLATE != "gather":
                        for k in range(K):
                            nc.gpsimd.indirect_dma_start(
                                out=G[:, k, :Din],
                                out_offset=None,
                                in_=table[:, :],
                                in_offset=bass.IndirectOffsetOnAxis(
                                    ap=gidx_sb[:, o + k:o + k + 1], axis=0
                                ),
                            )
                    A = tree_add(G, K, Din)

                    if ybatch is None:
                        ybatch = bpool.tile([P, nb, Dout],
                                            f32 if kind == 3 else bf16,
                                            tag=f"yb{kind}")
                        y0 = jj
                        ycnt = 0

                    if kind == 1:
                        ps_b = ppa.tile([P, P], f32, tag="pa")
                        nc.tensor.matmul(
                            out=ps_b[:, :D1],
                            lhsT=drownb[0:1, jj * P:(jj + 1) * P],
                            rhs=b1b[0:1, :], start=True, stop=True,
                        )
                        tmp = wpool.tile([P, D1], f32, tag="tmp1")
                        nc.vector.tensor_scalar_mul(
                            out=tmp[:, :], in0=A,
                            scalar1=d2own_sb[:, jj:jj + 1],
                        )
                        xh = wpool.tile([P, D1], f32, tag="xh")
                        nc.vector.tensor_tensor(
                            out=xh[:, :], in0=tmp[:, :], in1=ps_b[:, :D1],
                            op=add,
                        )
                        xhr = wpool.tile([P, D1], f32, tag="xhr")
                        nc.scalar.activation(
                            out=xhr[:, :], in_=xh[:, :],
                            func=mybir.ActivationFunctionType.Relu,
                        )
                        tps = ppa.tile([P, P], f32, tag="pa")
                        nc.tensor.transpose(
                            out=tps[:D1, :], in_=xhr[:, :],
                            identity=identf[:, :],
                        )
                        at = wpool.tile([P, P], bf16, tag="at")
                        nc.vector.tensor_copy(out=at[:D1, :], in_=tps[:D1, :])
                        z2 = ppb.tile([P, D2], f32, tag="pb")
                        nc.tensor.matmul(
                            out=z2[:, :], lhsT=at[:D1, :], rhs=W2b[:, :],
                            start=True, stop=True,
                        )
                        nc.vector.tensor_copy(
                            out=ybatch[:, ycnt, :], in_=z2[:, :]
                        )
                    elif kind == 2:
                        ps_b = ppb.tile([P, D2], f32, tag="pb")
                        nc.tensor.matmul(
                            out=ps_b[:, :],
                            lhsT=drownb[0:1, jj * P:(jj + 1) * P],
                            rhs=b2b[0:1, :], start=True, stop=True,
                        )
                        tmp = wpool.tile([P, D2], f32, tag="tmp2")
                        nc.vector.tensor_scalar_mul(
                            out=tmp[:, :], in0=A,
                            scalar1=d2own_sb[:, jj:jj + 1],
                        )
                        x3 = wpool.tile([P, D2], f32, tag="x3")
                        nc.vector.tensor_tensor(
                            out=x3[:, :], in0=tmp[:, :], in1=ps_b[:, :],
                            op=add,
                        )
                        tps = ppa.tile([P, P], f32, tag="pa")
                        nc.tensor.transpose(
                            out=tps[:D2, :], in_=x3[:, :],
                            identity=identf[:, :],
                        )
                        at = wpool.tile([P, P], bf16, tag="at")
                        nc.vector.tensor_copy(out=at[:D2, :], in_=tps[:D2, :])
                        z3 = ppb.tile([P, D2], f32, tag="pb")
                        nc.tensor.matmul(
                            out=z3[:, :D3], lhsT=at[:D2, :], rhs=W3b[:, :],
                            start=True, stop=True,
                        )
                        nc.vector.tensor_copy(
                            out=ybatch[:, ycnt, :], in_=z3[:, :D3]
                        )
                    else:
                        ps_b = ppb.tile([P, D2], f32, tag="pb")
                        nc.tensor.matmul(
                            out=ps_b[:, :D3], lhsT=onesb[0:1, :],
                            rhs=b3b[0:1, :], start=True, stop=True,
                        )
                        tmp = wpool.tile([P, D3], f32, tag="tmp3")
                        nc.vector.tensor_scalar_mul(
                            out=tmp[:, :], in0=A,
                            scalar1=d1own_sb[:, jj:jj + 1],
                        )
                        z3 = wpool.tile([P, D3], f32, tag="z3f")
                        nc.vector.tensor_tensor(
                            out=z3[:, :], in0=tmp[:, :], in1=ps_b[:, :D3],
                            op=add,
                        )
                        mneg = mpool.tile([P, 1], f32, tag="mneg")
                        nc.vector.tensor_reduce(
                            out=mneg[:, :], in_=z3[:, :],
                            axis=mybir.AxisListType.X,
                            op=mybir.AluOpType.max, negate=True,
                        )
                        ex = wpool.tile([P, D3], f32, tag="ex")
                        nc.scalar.activation(
                            out=ex[:, :], in_=z3[:, :],
                            func=mybir.ActivationFunctionType.Exp,
                            bias=mneg[:, :1],
                        )
                        ssum = mpool.tile([P, 1], f32, tag="ssum")
                        nc.vector.tensor_reduce(
                            out=ssum[:, :], in_=ex[:, :],
                            axis=mybir.AxisListType.X, op=add,
                        )
                        rec = mpool.tile([P, 1], f32, tag="rec")
                        nc.vector.reciprocal(out=rec[:, :], in_=ssum[:, :])
                        nc.vector.tensor_scalar_mul(
                            out=ybatch[:, ycnt, :], in0=ex[:, :],
                            scalar1=rec[:, :1],
                        )
                    ycnt += 1
                    full = ycnt == nb
                    chunk_end = jj + 1 in list(jj_cut[1:])
                    if full or chunk_end:
                        flush(ybatch, y0, ycnt)
                        ybatch = None
                    # AG after the last block of each chunk (kinds 1, 2)
                    if kind in (1, 2) and chunk_end and ABLATE != "ag":
                        c = int(chunk_of_jj[jj])
                        slc = (sl2 if kind == 1 else sl3)[c]
                        tbf = t2 if kind == 1 else t3
                        r0 = int(chunk_row_base[c])
                        r1 = int(chunk_row_base[c + 1])
                        nc.gpsimd.collective_compute(
                            "AllGather", mybir.AluOpType.bypass,
                            replica_groups=rg,
                            ins=[slc[:, :]], outs=[tbf[r0:r1, :]],
                        )
                if ybatch is not None:
                    flush(ybatch, y0, ycnt)

            layer(t1, D1, 1)
            layer(t2, D2, 2)
            layer(t3, D3, 3)

    nc.compile()
    return nc


# --------------------------------------------------------------------------
# Runner / entry point
# --------------------------------------------------------------------------
def build_all(edge_index, n, D0, D1, D2, D3):
    pre = preprocess5(edge_index, n)
    nc = build_bass5(pre, D0, D1, D2, D3)
    return pre, nc


def shard_inputs(pre, x, W1, b1, W2, b2, W3, b3):
    try:
        import ml_dtypes
        bfdt = ml_dtypes.bfloat16
    except ImportError:
        bfdt = None
    n, D0 = x.shape
    D1, D2, D3 = W1.shape[1], W2.shape[1], W3.shape[1]
    n_pad = pre["n_pad"]
    x_pad = np.zeros((n_pad, D0), np.float32)
    x_pad[pre["row_of_node"]] = np.asarray(x, np.float32)
    x_T = np.ascontiguousarray(x_pad.T)
    x_T = x_T.astype(bfdt) if bfdt is not None else x_T.astype(np.float32)

    in_maps = []
    for c in range(N_CORES):
        in_maps.append({
            "x_T": x_T,
            "gidx": np.ascontiguousarray(pre["gidx"][c]),
            "d1all": pre["dinv1_all"],
            "d1own": np.ascontiguousarray(pre["dinv1_own"][c]),
            "d2own": np.ascontiguousarray(pre["dinv2_own"][c]),
            "drown": (np.ascontiguousarray(pre["dinvr_own"][c]).astype(bfdt)
                      if bfdt is not None else
                      np.ascontiguousarray(pre["dinvr_own"][c])),
            "W1": np.asarray(W1, np.float32),
            "W2": np.asarray(W2, np.float32),
            "W3": np.asarray(W3, np.float32),
            "b1": np.asarray(b1, np.float32).reshape(1, D1),
            "b2": np.asarray(b2, np.float32).reshape(1, D2),
            "b3": np.asarray(b3, np.float32).reshape(1, D3),
        })
    return in_maps


def unshard(pre, results):
    out_all = np.stack([np.asarray(results[c]["out"]) for c in range(N_CORES)])
    n_pad = pre["n_pad"]
    core_r = np.zeros(n_pad, np.int64)
    loc_r = np.zeros(n_pad, np.int64)
    for c in range(N_CORES):
        core_r[pre["tbl_row_of"][c]] = c
        loc_r[pre["tbl_row_of"][c]] = np.arange(pre["J"] * P)
    rows = pre["row_of_node"]
    return out_all[core_r[rows], loc_r[rows]]


def make_runner(nc, n_cores=N_CORES):
    """Build the shard_map'd executable once; return (run_fn, time_fn)."""
    import jax
    import numpy as np2
    from jax.sharding import Mesh, PartitionSpec, NamedSharding
    from jax.experimental.shard_map import shard_map
    import concourse.mybir as mybir
    from concourse import bass2jax

    bass2jax.install_neuronx_cc_hook()

    in_names, out_names, out_avals, zero_outs = [], [], [], []
    for alloc in nc.m.functions[0].allocations:
        if not isinstance(alloc, mybir.MemoryLocationSet):
            continue
        name = alloc.memorylocations[0].name
        if alloc.kind == "ExternalInput":
            in_names.append(name)
        elif alloc.kind == "ExternalOutput":
            out_names.append(name)
            shape = tuple(alloc.tensor_shape)
            dtype = mybir.dt.np(alloc.dtype)
            out_avals.append(jax.core.ShapedArray(shape, dtype))
            zero_outs.append(np2.zeros(shape, dtype))
    partition_name = nc.partition_id_tensor.name if nc.partition_id_tensor else None
    if partition_name is not None and partition_name in in_names:
        in_names.remove(partition_name)
    n_params = len(in_names)
    n_outs = len(out_avals)
    all_in_names = in_names + out_names
    if partition_name is not None:
        all_in_names = all_in_names + [partition_name]

    def _body(*args):
        operands = list(args)
        if partition_name is not None:
            operands.append(bass2jax.partition_id_tensor())
        outs = bass2jax._bass_exec_p.bind(
            *operands,
            out_avals=tuple(out_avals),
            in_names=tuple(all_in_names),
            out_names=tuple(out_names),
            lowering_input_output_aliases=(),
            sim_require_finite=True,
            sim_require_nnan=True,
            nc=nc,
        )
        return tuple(outs)

    devices = jax.devices()[:n_cores]
    mesh = Mesh(np2.asarray(devices), ("core",))
    in_specs = (PartitionSpec("core"),) * (n_params + n_outs)
    out_specs = (PartitionSpec("core"),) * n_outs
    donate = tuple(range(n_params, n_params + n_outs))
    sharded = jax.jit(
        shard_map(_body, mesh=mesh, in_specs=in_specs, out_specs=out_specs,
                  check_rep=False),
        donate_argnums=donate, keep_unused=True,
    )
    sh = NamedSharding(mesh, PartitionSpec("core"))

    def _concat_inputs(in_maps):
        return [
            np2.concatenate([np2.asarray(in_maps[c][nm]) for c in range(n_cores)], axis=0)
            for nm in in_names
        ]

    def _zeros():
        return [np2.zeros((n_cores * z.shape[0], *z.shape[1:]), z.dtype)
                for z in zero_outs]

    def run_fn(in_maps):
        out_arrs = sharded(*_concat_inputs(in_maps), *_zeros())
        return [
            {nm: np2.asarray(out_arrs[i]).reshape(n_cores, *out_avals[i].shape)[c]
             for i, nm in enumerate(out_names)}
            for c in range(n_cores)
        ]

    def time_fn(in_maps, iters=5):
        import time as _t
        dev_in = [jax.device_put(a, sh) for a in _concat_inputs(in_maps)]
        for a in dev_in:
            a.block_until_ready()
        times = []
        for _ in range(iters):
            zs = [jax.device_put(z, sh) for z in _zeros()]
            for z in zs:
                z.block_until_ready()
            t0 = _t.time()
            outs = sharded(*dev_in, *zs)
            for o in outs:
                o.block_until_ready()
            times.append(_t.time() - t0)
        return times

    return run_fn, time_fn


def kernel(x, edge_index, W1, b1, W2, b2, W3, b3, _trace=False, _timed=0):
    from concourse.bass_utils import run_bass_kernel_spmd

    x = np.asarray(x, dtype=np.float32)
    n, D0 = x.shape
    D1, D2, D3 = W1.shape[1], W2.shape[1], W3.shape[1]

    pre, nc = build_all(edge_index, n, D0, D1, D2, D3)
    in_maps = shard_inputs(pre, x, W1, b1, W2, b2, W3, b3)

    if _timed:
        run_fn, time_fn = make_runner(nc)
        results = run_fn(in_maps)
        times = time_fn(in_maps, _timed)
        full = unshard(pre, results)
        return full.astype(np.float32), times

    res = run_bass_kernel_spmd(
        nc, in_maps, core_ids=list(range(N_CORES)), trace=_trace
    )
    full = unshard(pre, res.results)
    return full.astype(np.float32)
